# revision 21
# baseline (speedup 1.0000x reference)
"""Transformer block (pre-LN, non-causal full softmax, no 1/sqrt(D) scaling)
on 8 TRN2 NeuronCores.

v2 design (no collective): core c owns batch b = c//2 and query-token half
q = c%2. The host rolls each batch's token axis so the core's query half is
rows 0:512 (full non-causal attention is permutation-invariant over kv
tokens). Each core LayerNorms + projects K/V for ALL 1024 tokens of its
batch (duplicating its pair core's K/V work, ~27us of PE) which removes the
AllGather + barrier (~45us of dead time in v1) and makes every core fully
independent.

Key structures:
- LN gamma is folded into Wq/Wk/Wv/W1 host-side; LN beta becomes per-feature
  bias vectors applied in the Q/K psum->sbuf copies (per-partition bias) and
  folded into bp/b1 for the V/FFN paths (softmax weights sum to 1, so the
  V-bias contribution is exactly vb @ Wp added to bp).
- V is stored token-major with a 65-column head stride whose last column is
  1.0 (tile pre-memset to 1.0, psum copied through a strided AP); PV matmuls
  use M=65 so softmax denominators accumulate for free in psum row 64 --
  this deletes v1's 16 ones-matmuls per head pair (27us of PE).
- Scores use 2-bank [128,1024] psum tiles so exp runs as 8 (not 16)
  activations per pair; normalization = DVE copy of row 64, fast reciprocal,
  gpsimd partition_broadcast (out must be partition-base 0), DVE STT.
- Attn-out proj and FFN2 are "flipped" (activation chunks as lhsT, weights
  as rhs) so psum comes out token-major: residual adds fuse directly and the
  output streams to DRAM per 128x512 chunk with no transposes.
- Weights host-cast bf16; activations bf16; accumulation fp32 in PSUM.
HW constraints honored: bf16 tolerates partition offsets; only one DVE
operand may read PSUM; partition_broadcast outputs must start at partition 0.
"""

import ml_dtypes
import numpy as np

import concourse.bass as bass
import concourse.mybir as mybir
import concourse.tile as tile
from concourse import bacc
from concourse.bass_utils import run_bass_kernel_spmd

F32 = mybir.dt.float32
BF16 = mybir.dt.bfloat16
FP8 = mybir.dt.float8e4
W1SCALE = 64.0
AF = mybir.ActivationFunctionType
ALU = mybir.AluOpType

B, T, E, H, D, FF = 4, 1024, 1024, 16, 64, 4096
TQ = 512          # own query tokens per core
TK = 1024         # kv tokens (full batch)
NCORES = 8
EPS = 1e-5
P = 128

_CACHE: dict = {}


def _emit(nc, tc, d, out_d):
    const_cm = tc.tile_pool(name="const", bufs=1, side="right")
    const = const_cm.__enter__()
    eye = const.tile([P, P], BF16)
    nc.sync.dma_start(out=eye[:], in_=d["eye"][:, :])
    eye8 = const.tile([P, P], FP8)
    nc.sync.dma_start(out=eye8[:], in_=d["eye8"][:, :])
    epst = const.tile([P, 1], F32)
    nc.vector.memset(epst[:], EPS)
    bias = {}
    for name, w in [("qb", 8), ("kb", 8), ("b1", 32)]:
        bias[name] = const.tile([P, w], F32, tag=f"bias_{name}", name=f"bias_{name}")
        nc.sync.dma_start(out=bias[name][:], in_=d[name][:, :])
    bpb = const.tile([P, E], F32, tag="bpb", name="bpb")
    nc.sync.dma_start(out=bpb[:], in_=d["bpb"][:, :])

    # ---- long-lived pools. Right-side LIFO: const -> hT -> wkq -> kv ----
    hT_cm = tc.tile_pool(name="hTp", side="right", bufs=1)
    hTp = hT_cm.__enter__()
    hT = [hTp.tile([P, TK], BF16, tag=f"hT{j}", name=f"hT{j}") for j in range(8)]

    wkq_cm = tc.tile_pool(name="wkq", side="right", bufs=1)
    wkqp = wkq_cm.__enter__()
    wks = [wkqp.tile([P, E], BF16, tag=f"wk{k}", name=f"wk{k}") for k in range(8)]
    wqs = [wkqp.tile([P, E], BF16, tag=f"wq{k}", name=f"wq{k}") for k in range(8)]

    kv_cm = tc.tile_pool(name="kvp", bufs=1, side="right")
    kvp = kv_cm.__enter__()
    ktc = [kvp.tile([P, TK], BF16, tag=f"kt{m}", name=f"kt{m}") for m in range(8)]
    qtc = [kvp.tile([P, TQ], BF16, tag=f"qt{m}", name=f"qt{m}") for m in range(8)]
    vv = [kvp.tile([P, 16 * 65], BF16, tag=f"v{j}", name=f"v{j}") for j in range(8)]
    for j in range(8):
        nc.gpsimd.memset(vv[j][:], 1.0)

    # ---- stage 1: LN1 (gamma/beta folded into weights) + transpose to h^T,
    # with the V projection of token-chunk i interleaved right behind chunk
    # i's LayerNorm so the PE ramps early and stays busy.
    with tc.tile_pool(name="wv_s", side="left", bufs=1) as wvp, \
         tc.tile_pool(name="s1x", side="left", bufs=8) as xp, \
         tc.tile_pool(name="s1s", side="left", bufs=6) as sp, \
         tc.tile_pool(name="s1ps", bufs=4, space="PSUM") as tpp, \
         tc.tile_pool(name="ps_v", bufs=4, space="PSUM") as vpp:
        xts = [xp.tile([P, E], BF16, tag="xt", name=f"xt{i}") for i in range(8)]
        # DMA queue order: first LN chunk, then wv (V proj needs it at ~4us),
        # then the rest of x (bf16 halves the critical startup DMA; the f32 x
        # is still used for the exact residual adds), then wk/wq.
        nc.sync.dma_start(out=xts[0][:], in_=d["xbf"][0:P, :])
        wvs = []
        for k in range(8):
            w = wvp.tile([P, E], BF16, tag=f"wv{k}", name=f"wv{k}")
            nc.sync.dma_start(out=w[:], in_=d["wv"][k * P:(k + 1) * P, :])
            wvs.append(w)
        for i in range(1, 8):
            nc.sync.dma_start(out=xts[i][:], in_=d["xbf"][i * P:(i + 1) * P, :])
        for k in range(8):
            nc.sync.dma_start(out=wks[k][:], in_=d["wk"][k * P:(k + 1) * P, :])
        for k in range(8):
            nc.sync.dma_start(out=wqs[k][:], in_=d["wq"][k * P:(k + 1) * P, :])

        for i in range(8):
            xt = xts[i]
            stats = sp.tile([P, 2, 6], F32, tag="stats")
            nc.vector.bn_stats(stats[:, 0, :], xt[:, 0:512])
            nc.vector.bn_stats(stats[:, 1, :], xt[:, 512:1024])
            mv = sp.tile([P, 2], F32, tag="mv")
            nc.vector.bn_aggr(mv[:], stats[:])
            rsig = sp.tile([P, 1], F32, tag="rsig")
            nc.scalar.activation(rsig[:], mv[:, 1:2], AF.Sqrt, bias=epst[:])
            nc.vector.reciprocal(rsig[:], rsig[:])
            nmr = sp.tile([P, 1], F32, tag="nmr")
            nc.vector.scalar_tensor_tensor(nmr[:], mv[:, 0:1], -1.0, rsig[:],
                                           ALU.mult, ALU.mult)
            xn = xp.tile([P, E], BF16, tag="xn")
            nc.scalar.activation(xn[:], xt[:], AF.Identity,
                                 bias=nmr[:], scale=rsig[:])
            for j in range(8):
                pt = tpp.tile([P, P], BF16, tag="tp")
                nc.tensor.transpose(pt[:], xn[:, j * P:(j + 1) * P], eye[:])
                dst = hT[j][:, i * P:(i + 1) * P]
                if j in (0, 2, 4, 6, 7):
                    nc.scalar.copy(dst, pt[:])
                else:
                    nc.vector.tensor_copy(dst, pt[:])
            # V proj of token-chunk i (token-major, 65-col head stride)
            for n in range(2):
                ps = vpp.tile([P, 512], F32, tag="psv")
                for k in range(8):
                    nc.tensor.matmul(ps[:], hT[k][:, i * P:(i + 1) * P],
                                     wvs[k][:, n * 512:(n + 1) * 512],
                                     start=(k == 0), stop=(k == 7))
                dst = vv[i][:].rearrange("p (h e) -> p h e", h=16)[:, 8 * n:8 * n + 8, 0:64]
                src = ps[:].rearrange("p (h e) -> p h e", h=8)
                if n == 0:
                    nc.scalar.activation(dst, src, AF.Identity)
                else:
                    nc.vector.tensor_copy(dst, src)

    # ---- stage 3: attention. K^T/Q^T projections of pair p+2 are emitted as
    # PE "filler" work inside pair p's score/exp stream so the tensor engine
    # never idles waiting on the scalar-engine exp (which would drop the PE
    # out of its max p-state clock).
    # w1 pool opens first (lives through FFN2) so left-side pool order is LIFO
    w1_cm = tc.tile_pool(name="w1s", side="left", bufs=1)
    w1p = w1_cm.__enter__()

    def load_w1_group(g):
        ws = []
        for k in range(8):
            w = w1p.tile([64, 2, 1024], FP8, tag=f"w1_{k}", bufs=2,
                         name=f"w1_{k}")
            nc.sync.dma_start(
                out=w[:], in_=d["w18"][:, :, k, g * 1024:(g + 1) * 1024])
            ws.append(w)
        return ws

    # prefetch FFN1 group-0 weights during attention
    w1g0 = load_w1_group(0)

    ot_cm = tc.tile_pool(name="otp", side="left", bufs=1)
    otp = ot_cm.__enter__()
    otc = [otp.tile([P, TQ], BF16, tag=f"ot{p}", name=f"ot{p}") for p in range(8)]
    wps = [otp.tile([P, E], BF16, tag=f"wp{p}", name=f"wp{p}") for p in range(8)]

    with tc.tile_pool(name="att_pt", side="left", bufs=18) as ptp, \
         tc.tile_pool(name="att_sc", side="left", bufs=1) as scp, \
         tc.tile_pool(name="ps_s", bufs=2, space="PSUM") as spp, \
         tc.tile_pool(name="ps_o", bufs=1, space="PSUM") as opp, \
         tc.tile_pool(name="ps_kq", bufs=2, space="PSUM") as kqp:
        state = {}
        pvps = {}

        def kq_thunks(m):
            # 24 matmul thunks projecting K^T (both halves) + Q^T for chunk m
            thunks = []
            holder = {}

            def mk(kind, half, k):
                def t():
                    if k == 0:
                        holder[(kind, half)] = kqp.tile([P, 512], F32,
                                                        tag="pkq", name="pkq")
                    ps = holder[(kind, half)]
                    if kind == "k":
                        nc.tensor.matmul(ps[:], wks[k][:, m * P:(m + 1) * P],
                                         hT[k][:, half * 512:(half + 1) * 512],
                                         start=(k == 0), stop=(k == 7),
                                         skip_group_check=True)
                        if k == 7:
                            nc.vector.tensor_scalar(
                                ktc[m][:, half * 512:(half + 1) * 512], ps[:],
                                bias["kb"][:, m:m + 1], 1.0,
                                ALU.add, op1=ALU.mult)
                    else:
                        nc.tensor.matmul(ps[:], wqs[k][:, m * P:(m + 1) * P],
                                         hT[k][:, 0:512],
                                         start=(k == 0), stop=(k == 7),
                                         skip_group_check=True)
                        if k == 7:
                            nc.vector.tensor_scalar(qtc[m][:], ps[:],
                                                    bias["qb"][:, m:m + 1], 1.0,
                                                    ALU.add, op1=ALU.mult)
                return t

            for half in range(2):
                for k in range(8):
                    thunks.append(mk("k", half, k))
            for k in range(8):
                thunks.append(mk("q", 0, k))
            return thunks

        def pv_thunks(p):
            ptj = state.pop(p)
            thunks = []

            def mk(j):
                def t():
                    if j == 0:
                        pvps[p] = (opp.tile([65, 512], F32, tag="psA", name="psA"),
                                   opp.tile([65, 512], F32, tag="psB", name="psB"))
                    psA, psB = pvps[p]
                    g, half = j // 2, j % 2
                    hA, hB = 2 * p, 2 * p + 1
                    nc.tensor.matmul(psA[:], vv[j][:, 65 * hA:65 * hA + 65],
                                     ptj[g][:, half * 512:(half + 1) * 512],
                                     start=(j == 0), stop=(j == 7),
                                     skip_group_check=True)
                    nc.tensor.matmul(psB[:], vv[j][:, 65 * hB:65 * hB + 65],
                                     ptj[4 + g][:, half * 512:(half + 1) * 512],
                                     start=(j == 0), stop=(j == 7),
                                     skip_group_check=True)
                return t

            for j in range(8):
                thunks.append(mk(j))
            return thunks

        def pv_finish(p):
            psA, psB = pvps.pop(p)
            dnA = scp.tile([1, 512], F32, tag="dnA", name="dnA")
            dnB = scp.tile([1, 512], F32, tag="dnB", name="dnB")
            nc.vector.tensor_copy(dnA[:], psA[64:65, :])
            nc.vector.tensor_copy(dnB[:], psB[64:65, :])
            rcA = scp.tile([1, 512], F32, tag="rcA", name="rcA")
            rcB = scp.tile([1, 512], F32, tag="rcB", name="rcB")
            nc.vector.reciprocal_approx_fast(rcA[:], dnA[:])
            nc.vector.reciprocal_approx_fast(rcB[:], dnB[:])
            bcA = scp.tile([64, 512], F32, tag="bcA", name="bcA")
            bcB = scp.tile([64, 512], F32, tag="bcB", name="bcB")
            nc.gpsimd.partition_broadcast(bcA[:], rcA[0:1, :], 64)
            nc.gpsimd.partition_broadcast(bcB[:], rcB[0:1, :], 64)
            nc.vector.scalar_tensor_tensor(otc[p][0:64, :], psA[0:64, :], 1.0,
                                           bcA[:], ALU.mult, ALU.mult)
            nc.vector.scalar_tensor_tensor(otc[p][64:128, :], psB[0:64, :], 1.0,
                                           bcB[:], ALU.mult, ALU.mult)

        # prologue: project K/Q for pairs 0 and 1
        for t in kq_thunks(0) + kq_thunks(1):
            t()

        for p in range(8):
            nc.sync.dma_start(out=wps[p][:], in_=d["wp"][p * P:(p + 1) * P, :])
            fillers = []
            if p + 2 <= 7:
                fillers += kq_thunks(p + 2)
            if p >= 1:
                fillers += pv_thunks(p - 1)
            ptj = []
            nf = 0
            for g in range(8):
                h, gg = g // 4, g % 4
                ps2 = spp.tile([P, 1024], F32, tag="ps2", name="ps2")
                for half in range(2):
                    cj = 2 * gg + half
                    nc.tensor.matmul(
                        ps2[:, half * 512:(half + 1) * 512],
                        ktc[p][64 * h:64 * h + 64, cj * P:(cj + 1) * P],
                        qtc[p][64 * h:64 * h + 64, :],
                        start=True, stop=True, skip_group_check=True)
                pt2 = ptp.tile([P, 1024], BF16, tag="pt", name="pt2")
                nc.scalar.activation(pt2[:], ps2[:], AF.Exp)
                ptj.append(pt2)
                # interleave filler matmuls to keep the PE dense while the
                # scalar engine drains the exp queue
                take = (len(fillers) * (g + 1)) // 8 - nf
                for t in fillers[nf:nf + take]:
                    t()
                nf += take
            state[p] = ptj
            if p >= 1:
                pv_finish(p - 1)
        for t in pv_thunks(7):
            t()
        pv_finish(7)

    kv_cm.__exit__(None, None, None)
    wkq_cm.__exit__(None, None, None)
    hT_cm.__exit__(None, None, None)

    # ---- stage 4: flipped attn-out proj (token-major psum) + residual + LN2 ----
    x2s_cm = tc.tile_pool(name="x2s", bufs=1, side="right")
    x2s = x2s_cm.__enter__()
    x2 = [x2s.tile([P, E], F32, tag=f"x2_{i}", name=f"x2_{i}") for i in range(4)]
    h2dr = [x2s.tile([64, 2, TQ], FP8, tag=f"h2dr{j}", name=f"h2dr{j}")
            for j in range(8)]
    b2b = x2s.tile([P, E], F32, tag="b2b", name="b2b")
    nc.sync.dma_start(out=b2b[:], in_=d["b2b"][:, :])

    with tc.tile_pool(name="s4x", side="left", bufs=2) as xqp, \
         tc.tile_pool(name="s4s", side="left", bufs=6) as sp, \
         tc.tile_pool(name="ps_pj", bufs=3, space="PSUM") as ppp, \
         tc.tile_pool(name="ps_t4", bufs=4, space="PSUM") as tpp:
        for i in range(4):
            # xq = x(own) + bp_eff, prepared on gpsimd while PE runs proj
            xq = xqp.tile([P, E], F32, tag="xq")
            nc.sync.dma_start(out=xq[:], in_=d["x"][i * P:(i + 1) * P, :])
            nc.vector.scalar_tensor_tensor(xq[:], xq[:], 1.0, bpb[:],
                                           ALU.mult, ALU.add)
            for n in range(2):
                pj = ppp.tile([P, 512], F32, tag="pj", name="pj")
                for p in range(8):
                    nc.tensor.matmul(pj[:], otc[p][:, i * P:(i + 1) * P],
                                     wps[p][:, n * 512:(n + 1) * 512],
                                     start=(p == 0), stop=(p == 7))
                nc.vector.scalar_tensor_tensor(
                    x2[i][:, n * 512:(n + 1) * 512], pj[:], 1.0,
                    xq[:, n * 512:(n + 1) * 512], ALU.mult, ALU.add)
            stats = sp.tile([P, 2, 6], F32, tag="stats")
            nc.vector.bn_stats(stats[:, 0, :], x2[i][:, 0:512])
            nc.vector.bn_stats(stats[:, 1, :], x2[i][:, 512:1024])
            mv = sp.tile([P, 2], F32, tag="mv")
            nc.vector.bn_aggr(mv[:], stats[:])
            rsig = sp.tile([P, 1], F32, tag="rsig")
            nc.scalar.activation(rsig[:], mv[:, 1:2], AF.Sqrt, bias=epst[:])
            nc.vector.reciprocal(rsig[:], rsig[:])
            nmr = sp.tile([P, 1], F32, tag="nmr")
            nc.vector.scalar_tensor_tensor(nmr[:], mv[:, 0:1], -1.0, rsig[:],
                                           ALU.mult, ALU.mult)
            xn = sp.tile([P, E], FP8, tag="xn")
            nc.scalar.activation(xn[:], x2[i][:], AF.Identity,
                                 bias=nmr[:], scale=rsig[:])
            for j in range(8):
                for kt in range(2):
                    pt8 = tpp.tile([64, 256], FP8, tag="tp")
                    dst2 = pt8[:].rearrange("p (n s) -> p n s", s=2)[:, :, 0:1]
                    nc.tensor.transpose(
                        dst2, xn[:, j * P + kt * 64:j * P + kt * 64 + 64],
                        eye8[:])
                    src2 = pt8[:].rearrange("p (n s) -> p n s", s=2)[:, :, 0]
                    dst = h2dr[j][:, kt, i * P:(i + 1) * P]
                    if (2 * j + kt) % 2 == 0:
                        nc.scalar.copy(dst, src2)
                    else:
                        nc.vector.tensor_copy(dst, src2)

    ot_cm.__exit__(None, None, None)

    # ---- stage 5: FFN1 (feature-major rr) ----
    rr_cm = tc.tile_pool(name="relu", side="left", bufs=1)
    rrp = rr_cm.__enter__()
    rr = [rrp.tile([P, TQ], BF16, tag=f"r{k}", name=f"r{k}") for k in range(32)]
    # stage all of W2 during FFN1
    w2_cm = tc.tile_pool(name="w2s", side="left", bufs=1)
    w2p = w2_cm.__enter__()
    w2s = []
    for k in range(32):
        w2t = w2p.tile([P, E], BF16, tag=f"w2_{k}", name=f"w2_{k}")
        nc.sync.dma_start(out=w2t[:], in_=d["w2"][k * P:(k + 1) * P, :])
        w2s.append(w2t)

    with tc.tile_pool(name="ps_f1", bufs=4, space="PSUM") as fpp:
        for g in range(4):
            ws = w1g0 if g == 0 else load_w1_group(g)
            for m in range(8):
                ps = fpp.tile([P, TQ], F32, tag="ps")
                for k in range(8):
                    nc.tensor.matmul(ps[:], ws[k][:, :, m * P:(m + 1) * P],
                                     h2dr[k][:, :, :],
                                     start=(k == 0), stop=(k == 7),
                                     perf_mode=mybir.MatmulPerfMode.DoubleRow)
                col = g * 8 + m
                nc.scalar.activation(rr[col][:], ps[:], AF.Relu,
                                     bias=bias["b1"][:, col:col + 1],
                                     scale=float(1.0 / W1SCALE))

    # ---- FFN2 flipped: token-major psum, fused residual + b2, streamed out ----
    with tc.tile_pool(name="outp", side="left", bufs=3) as outp, \
         tc.tile_pool(name="ps_f2", bufs=3, space="PSUM") as f2pp:
        for i in range(4):
            for n in range(2):
                ps = f2pp.tile([P, 512], F32, tag="pf", name="pf")
                for k in range(32):
                    nc.tensor.matmul(ps[:], rr[k][:, i * P:(i + 1) * P],
                                     w2s[k][:, n * 512:(n + 1) * 512],
                                     start=(k == 0), stop=(k == 31))
                ot = outp.tile([P, 512], F32, tag="ot", name="ot")
                nc.vector.scalar_tensor_tensor(
                    ot[:], ps[:], 1.0, x2[i][:, n * 512:(n + 1) * 512],
                    ALU.mult, ALU.add)
                nc.vector.scalar_tensor_tensor(
                    ot[:], ot[:], 1.0, b2b[:, n * 512:(n + 1) * 512],
                    ALU.mult, ALU.add)
                nc.sync.dma_start(
                    out=out_d[i * P:(i + 1) * P, n * 512:(n + 1) * 512],
                    in_=ot[:])

    w2_cm.__exit__(None, None, None)
    rr_cm.__exit__(None, None, None)
    w1_cm.__exit__(None, None, None)
    x2s_cm.__exit__(None, None, None)
    const_cm.__exit__(None, None, None)


def _build():
    nc = bacc.Bacc("TRN2", target_bir_lowering=False, debug=False,
                   num_devices=NCORES)
    d = {}

    def din(name, shape, dt=F32):
        d[name] = nc.dram_tensor(name, shape, dt, kind="ExternalInput").ap()

    din("x", [TK, E], F32)
    din("xbf", [TK, E], BF16)
    for n in ("wq", "wk", "wv", "wp"):
        din(n, [E, E], BF16)
    din("w18", [64, 2, 8, FF], FP8)
    din("w2", [FF, E], BF16)
    din("eye", [P, P], BF16)
    din("eye8", [P, P], FP8)
    din("qb", [P, 8])
    din("kb", [P, 8])
    din("b1", [P, 32])
    din("bpb", [P, E])
    din("b2b", [P, E])
    out_d = nc.dram_tensor("out", [TQ, E], F32, kind="ExternalOutput").ap()
    with nc.allow_low_precision(reason="bf16 compute"):
        with tile.TileContext(nc) as tc:
            _emit(nc, tc, d, out_d)
    nc.compile()
    return nc


def _get_nc():
    if "nc" not in _CACHE:
        _CACHE["nc"] = _build()
    return _CACHE["nc"]


def _colmajor(v, width):
    return np.ascontiguousarray(np.asarray(v, np.float32).reshape(width, P).T)


def make_in_maps(x, ln1_g, ln1_b, Wq, Wk, Wv, Wp, bp, ln2_g, ln2_b,
                 W1, b1, W2, b2):
    x = np.asarray(x, dtype=np.float32)
    f32 = np.float32
    g1 = np.asarray(ln1_g, f32)
    b1n = np.asarray(ln1_b, f32)
    g2 = np.asarray(ln2_g, f32)
    b2n = np.asarray(ln2_b, f32)
    Wq_r = np.transpose(np.asarray(Wq, f32), (1, 0, 2)).reshape(E, E)
    Wk_r = np.transpose(np.asarray(Wk, f32), (1, 0, 2)).reshape(E, E)
    Wv_r = np.transpose(np.asarray(Wv, f32), (1, 0, 2)).reshape(E, E)
    Wp_ = np.asarray(Wp, f32)
    W1_ = np.asarray(W1, f32)
    qb = b1n @ Wq_r            # [E]
    kb = b1n @ Wk_r
    vb = b1n @ Wv_r
    bp_eff = np.asarray(bp, f32) + vb @ Wp_
    b1_eff = np.asarray(b1, f32) + b2n @ W1_
    shared = {
        "wq": np.ascontiguousarray(g1[:, None] * Wq_r).astype(ml_dtypes.bfloat16),
        "wk": np.ascontiguousarray(g1[:, None] * Wk_r).astype(ml_dtypes.bfloat16),
        "wv": np.ascontiguousarray(g1[:, None] * Wv_r).astype(ml_dtypes.bfloat16),
        "wp": Wp_.astype(ml_dtypes.bfloat16),
        "w18": np.ascontiguousarray(
            (g2[:, None] * W1_ * 64.0).reshape(8, 2, 64, FF)
            .transpose(2, 1, 0, 3)).astype(ml_dtypes.float8_e4m3fn),
        "w2": np.asarray(W2, f32).astype(ml_dtypes.bfloat16),
        "eye": np.eye(P, dtype=ml_dtypes.bfloat16),
        "eye8": np.eye(P, dtype=ml_dtypes.float8_e4m3fn),
        "qb": _colmajor(qb, 8),
        "kb": _colmajor(kb, 8),
        "b1": _colmajor(b1_eff, 32),
        "bpb": np.ascontiguousarray(np.broadcast_to(bp_eff, (P, E))).astype(f32),
        "b2b": np.ascontiguousarray(
            np.broadcast_to(np.asarray(b2, f32), (P, E))).astype(f32),
    }
    in_maps = []
    for c in range(NCORES):
        b = c // 2
        q0 = TQ * (c % 2)
        xb = x[b]
        x_roll = np.ascontiguousarray(np.concatenate([xb[q0:], xb[:q0]], axis=0))
        in_maps.append({"x": x_roll,
                        "xbf": x_roll.astype(ml_dtypes.bfloat16), **shared})
    return in_maps


def assemble_out(results):
    out = np.empty((B, T, E), dtype=np.float32)
    for c in range(NCORES):
        b = c // 2
        q0 = TQ * (c % 2)
        out[b, q0:q0 + TQ] = results[c]["out"]
    return out


def kernel(x, ln1_g, ln1_b, Wq, Wk, Wv, Wp, bp, ln2_g, ln2_b, W1, b1, W2, b2,
           **_ignored):
    in_maps = make_in_maps(x, ln1_g, ln1_b, Wq, Wk, Wv, Wp, bp,
                           ln2_g, ln2_b, W1, b1, W2, b2)
    nc = _get_nc()
    res = run_bass_kernel_spmd(nc, in_maps, core_ids=list(range(NCORES)))
    return assemble_out(res.results)


# revision 22
# speedup vs baseline: 1.2011x; 1.2011x over previous
"""Transformer block (pre-LN, non-causal full softmax, no 1/sqrt(D) scaling)
on 8 TRN2 NeuronCores.

Sharding (no collectives): core c owns batch b = c//2 and query-token half
q = c%2. The host rolls each batch's token axis so the core's query half is
rows 0:512 (full non-causal attention is permutation-invariant over kv
tokens). Each core LayerNorms + projects K/V for ALL 1024 tokens of its
batch, duplicating its pair core's K/V work (~27us of PE): measured on this
axon fabric, a pairwise AllGather of the halves costs ~40us of dead time
(fake_nrt host-emulated collectives, fixed ~115us completion), so full
duplication is strictly faster and makes every core independent.

Schedule: the tensor engine drops out of its max p-state clock (2.4 -> 1.2
GHz) whenever it idles, so the emission interleaves work to keep it dense:
- V projection of token-chunk i is emitted right behind chunk i's LayerNorm.
- K^T/Q^T projections of head-pair p+2 are spread between pair p's score
  matmuls as PE filler while the scalar engine drains the exp queue (the
  per-pair exp stream, 8x [128,1024] activations, is the co-pacing engine).

Data layout / algebraic tricks:
- LN gamma folds into Wq/Wk/Wv/W1 host-side; LN beta becomes per-feature
  bias vectors fused into the Q/K psum->sbuf copies (per-partition bias on
  DVE tensor_scalar) and folds exactly into bp/b1 for the V/FFN paths
  (softmax weights sum to 1, so the V-bias contribution is vb @ Wp).
- V is token-major with a 65-column head stride whose last column is 1.0
  (tile pre-memset, psum copied through a strided AP); PV matmuls use M=65
  so softmax denominators accumulate free in psum row 64 (kills the 16
  ones-matmuls per pair of the naive scheme).
- Scores use 2-bank [128,1024] psum tiles -> 8 exps/pair instead of 16.
- Softmax normalize: DVE copy of psum row 64, reciprocal_approx_fast
  (inputs must sit at partition base 0!), gpsimd partition_broadcast
  (output must be partition-base 0), DVE STT multiply.
- Attn-out proj and FFN2 are flipped (activation chunks as lhsT, weights as
  rhs) so psum is token-major: residuals fuse into the psum->sbuf STT and
  the output streams to DRAM per 128x512 chunk with no transposes.
- x is loaded bf16 for the LN path (halves startup DMA); the residual adds
  reload the f32 x during idle DMA windows for exactness.
- Weights bf16 (fp8 DoubleRow tried for FFN1: slower on HW and rel err
  1.9e-2, too close to the 2e-2 gate); activations bf16, accumulation f32.
"""

import ml_dtypes
import numpy as np

import concourse.bass as bass
import concourse.mybir as mybir
import concourse.tile as tile
from concourse import bacc
from concourse.bass_utils import run_bass_kernel_spmd

F32 = mybir.dt.float32
BF16 = mybir.dt.bfloat16
AF = mybir.ActivationFunctionType
ALU = mybir.AluOpType

B, T, E, H, D, FF = 4, 1024, 1024, 16, 64, 4096
TQ = 512          # own query tokens per core
TK = 1024         # kv tokens (full batch)
NCORES = 8
EPS = 1e-5
P = 128

_CACHE: dict = {}


def _emit(nc, tc, d, out_d):
    const_cm = tc.tile_pool(name="const", bufs=1, side="right")
    const = const_cm.__enter__()
    eye = const.tile([P, P], BF16)
    nc.sync.dma_start(out=eye[:], in_=d["eye"][:, :])
    epst = const.tile([P, 1], F32)
    nc.vector.memset(epst[:], EPS)
    bias = {}
    for name, w in [("qb", 8), ("kb", 8), ("b1", 32)]:
        bias[name] = const.tile([P, w], F32, tag=f"bias_{name}", name=f"bias_{name}")
        nc.sync.dma_start(out=bias[name][:], in_=d[name][:, :])
    bpb = const.tile([P, E], F32, tag="bpb", name="bpb")
    nc.sync.dma_start(out=bpb[:], in_=d["bpb"][:, :])

    # ---- long-lived pools. Right-side LIFO: const -> hT -> wkq -> kv ----
    hT_cm = tc.tile_pool(name="hTp", side="right", bufs=1)
    hTp = hT_cm.__enter__()
    hT = [hTp.tile([P, TK], BF16, tag=f"hT{j}", name=f"hT{j}") for j in range(8)]

    wkq_cm = tc.tile_pool(name="wkq", side="right", bufs=1)
    wkqp = wkq_cm.__enter__()
    wks = [wkqp.tile([P, E], BF16, tag=f"wk{k}", name=f"wk{k}") for k in range(8)]
    wqs = [wkqp.tile([P, E], BF16, tag=f"wq{k}", name=f"wq{k}") for k in range(8)]

    kv_cm = tc.tile_pool(name="kvp", bufs=1, side="right")
    kvp = kv_cm.__enter__()
    ktc = [kvp.tile([P, TK], BF16, tag=f"kt{m}", name=f"kt{m}") for m in range(8)]
    qtc = [kvp.tile([P, TQ], BF16, tag=f"qt{m}", name=f"qt{m}") for m in range(8)]
    vv = [kvp.tile([P, 16 * 65], BF16, tag=f"v{j}", name=f"v{j}") for j in range(8)]
    for j in range(8):
        nc.gpsimd.memset(vv[j][:], 1.0)

    # ---- stage 1: LN1 (gamma/beta folded into weights) + transpose to h^T,
    # with the V projection of token-chunk i interleaved right behind chunk
    # i's LayerNorm so the PE ramps early and stays busy.
    with tc.tile_pool(name="wv_s", side="left", bufs=1) as wvp, \
         tc.tile_pool(name="s1x", side="left", bufs=8) as xp, \
         tc.tile_pool(name="s1s", side="left", bufs=6) as sp, \
         tc.tile_pool(name="s1ps", bufs=4, space="PSUM") as tpp, \
         tc.tile_pool(name="ps_v", bufs=4, space="PSUM") as vpp:
        xts = [xp.tile([P, E], BF16, tag="xt", name=f"xt{i}") for i in range(8)]
        # DMA queue order: first LN chunk, then wv (V proj needs it at ~4us),
        # then the rest of x (bf16 halves the critical startup DMA; the f32 x
        # is still used for the exact residual adds), then wk/wq.
        nc.sync.dma_start(out=xts[0][:], in_=d["xbf"][0:P, :])
        wvs = []
        for k in range(8):
            w = wvp.tile([P, E], BF16, tag=f"wv{k}", name=f"wv{k}")
            nc.sync.dma_start(out=w[:], in_=d["wv"][k * P:(k + 1) * P, :])
            wvs.append(w)
        for i in range(1, 8):
            nc.sync.dma_start(out=xts[i][:], in_=d["xbf"][i * P:(i + 1) * P, :])
        for k in range(8):
            nc.sync.dma_start(out=wks[k][:], in_=d["wk"][k * P:(k + 1) * P, :])
        for k in range(8):
            nc.sync.dma_start(out=wqs[k][:], in_=d["wq"][k * P:(k + 1) * P, :])

        for i in range(8):
            xt = xts[i]
            stats = sp.tile([P, 2, 6], F32, tag="stats")
            nc.vector.bn_stats(stats[:, 0, :], xt[:, 0:512])
            nc.vector.bn_stats(stats[:, 1, :], xt[:, 512:1024])
            mv = sp.tile([P, 2], F32, tag="mv")
            nc.vector.bn_aggr(mv[:], stats[:])
            rsig = sp.tile([P, 1], F32, tag="rsig")
            nc.scalar.activation(rsig[:], mv[:, 1:2], AF.Sqrt, bias=epst[:])
            nc.vector.reciprocal(rsig[:], rsig[:])
            nmr = sp.tile([P, 1], F32, tag="nmr")
            nc.vector.scalar_tensor_tensor(nmr[:], mv[:, 0:1], -1.0, rsig[:],
                                           ALU.mult, ALU.mult)
            xn = xp.tile([P, E], BF16, tag="xn")
            nc.scalar.activation(xn[:], xt[:], AF.Identity,
                                 bias=nmr[:], scale=rsig[:])
            for j in range(8):
                pt = tpp.tile([P, P], BF16, tag="tp")
                nc.tensor.transpose(pt[:], xn[:, j * P:(j + 1) * P], eye[:])
                dst = hT[j][:, i * P:(i + 1) * P]
                if j in (0, 2, 4, 6, 7):
                    nc.scalar.copy(dst, pt[:])
                else:
                    nc.vector.tensor_copy(dst, pt[:])
            # V proj of token-chunk i (token-major, 65-col head stride)
            for n in range(2):
                ps = vpp.tile([P, 512], F32, tag="psv")
                for k in range(8):
                    nc.tensor.matmul(ps[:], hT[k][:, i * P:(i + 1) * P],
                                     wvs[k][:, n * 512:(n + 1) * 512],
                                     start=(k == 0), stop=(k == 7))
                dst = vv[i][:].rearrange("p (h e) -> p h e", h=16)[:, 8 * n:8 * n + 8, 0:64]
                src = ps[:].rearrange("p (h e) -> p h e", h=8)
                if n == 0:
                    nc.scalar.activation(dst, src, AF.Identity)
                else:
                    nc.vector.tensor_copy(dst, src)

    # ---- stage 3: attention. K^T/Q^T projections of pair p+2 are emitted as
    # PE "filler" work inside pair p's score/exp stream so the tensor engine
    # never idles waiting on the scalar-engine exp (which would drop the PE
    # out of its max p-state clock).
    # w1 pool opens first (lives through FFN2) so left-side pool order is LIFO
    w1_cm = tc.tile_pool(name="w1s", side="left", bufs=1)
    w1p = w1_cm.__enter__()

    def load_w1_group(g):
        ws = []
        for k in range(8):
            w = w1p.tile([P, 1024], BF16, tag=f"w1_{k}", bufs=2, name=f"w1_{k}")
            nc.sync.dma_start(
                out=w[:], in_=d["w1"][k * P:(k + 1) * P,
                                      g * 1024:(g + 1) * 1024])
            ws.append(w)
        return ws

    # prefetch FFN1 group-0 weights during attention
    w1g0 = load_w1_group(0)

    ot_cm = tc.tile_pool(name="otp", side="left", bufs=1)
    otp = ot_cm.__enter__()
    otc = [otp.tile([P, TQ], BF16, tag=f"ot{p}", name=f"ot{p}") for p in range(8)]
    wps = [otp.tile([P, E], BF16, tag=f"wp{p}", name=f"wp{p}") for p in range(8)]

    with tc.tile_pool(name="att_pt", side="left", bufs=18) as ptp, \
         tc.tile_pool(name="att_sc", side="left", bufs=1) as scp, \
         tc.tile_pool(name="ps_s", bufs=2, space="PSUM") as spp, \
         tc.tile_pool(name="ps_o", bufs=1, space="PSUM") as opp, \
         tc.tile_pool(name="ps_kq", bufs=2, space="PSUM") as kqp:
        state = {}
        pvps = {}

        def kq_thunks(m):
            # 24 matmul thunks projecting K^T (both halves) + Q^T for chunk m
            thunks = []
            holder = {}

            def mk(kind, half, k):
                def t():
                    if k == 0:
                        holder[(kind, half)] = kqp.tile([P, 512], F32,
                                                        tag="pkq", name="pkq")
                    ps = holder[(kind, half)]
                    if kind == "k":
                        nc.tensor.matmul(ps[:], wks[k][:, m * P:(m + 1) * P],
                                         hT[k][:, half * 512:(half + 1) * 512],
                                         start=(k == 0), stop=(k == 7),
                                         skip_group_check=True)
                        if k == 7:
                            nc.vector.tensor_scalar(
                                ktc[m][:, half * 512:(half + 1) * 512], ps[:],
                                bias["kb"][:, m:m + 1], 1.0,
                                ALU.add, op1=ALU.mult)
                    else:
                        nc.tensor.matmul(ps[:], wqs[k][:, m * P:(m + 1) * P],
                                         hT[k][:, 0:512],
                                         start=(k == 0), stop=(k == 7),
                                         skip_group_check=True)
                        if k == 7:
                            nc.vector.tensor_scalar(qtc[m][:], ps[:],
                                                    bias["qb"][:, m:m + 1], 1.0,
                                                    ALU.add, op1=ALU.mult)
                return t

            for half in range(2):
                for k in range(8):
                    thunks.append(mk("k", half, k))
            for k in range(8):
                thunks.append(mk("q", 0, k))
            return thunks

        def pv_thunks(p):
            ptj = state.pop(p)
            thunks = []

            def mk(j):
                def t():
                    if j == 0:
                        pvps[p] = (opp.tile([65, 512], F32, tag="psA", name="psA"),
                                   opp.tile([65, 512], F32, tag="psB", name="psB"))
                    psA, psB = pvps[p]
                    g, half = j // 2, j % 2
                    hA, hB = 2 * p, 2 * p + 1
                    nc.tensor.matmul(psA[:], vv[j][:, 65 * hA:65 * hA + 65],
                                     ptj[g][:, half * 512:(half + 1) * 512],
                                     start=(j == 0), stop=(j == 7),
                                     skip_group_check=True)
                    nc.tensor.matmul(psB[:], vv[j][:, 65 * hB:65 * hB + 65],
                                     ptj[4 + g][:, half * 512:(half + 1) * 512],
                                     start=(j == 0), stop=(j == 7),
                                     skip_group_check=True)
                return t

            for j in range(8):
                thunks.append(mk(j))
            return thunks

        def pv_finish(p):
            psA, psB = pvps.pop(p)
            dnA = scp.tile([1, 512], F32, tag="dnA", name="dnA")
            dnB = scp.tile([1, 512], F32, tag="dnB", name="dnB")
            nc.vector.tensor_copy(dnA[:], psA[64:65, :])
            nc.vector.tensor_copy(dnB[:], psB[64:65, :])
            rcA = scp.tile([1, 512], F32, tag="rcA", name="rcA")
            rcB = scp.tile([1, 512], F32, tag="rcB", name="rcB")
            nc.vector.reciprocal_approx_fast(rcA[:], dnA[:])
            nc.vector.reciprocal_approx_fast(rcB[:], dnB[:])
            bcA = scp.tile([64, 512], F32, tag="bcA", name="bcA")
            bcB = scp.tile([64, 512], F32, tag="bcB", name="bcB")
            nc.gpsimd.partition_broadcast(bcA[:], rcA[0:1, :], 64)
            nc.gpsimd.partition_broadcast(bcB[:], rcB[0:1, :], 64)
            nc.vector.scalar_tensor_tensor(otc[p][0:64, :], psA[0:64, :], 1.0,
                                           bcA[:], ALU.mult, ALU.mult)
            nc.vector.scalar_tensor_tensor(otc[p][64:128, :], psB[0:64, :], 1.0,
                                           bcB[:], ALU.mult, ALU.mult)

        # prologue: project K/Q for pairs 0 and 1
        for t in kq_thunks(0) + kq_thunks(1):
            t()

        for p in range(8):
            nc.sync.dma_start(out=wps[p][:], in_=d["wp"][p * P:(p + 1) * P, :])
            fillers = []
            if p + 2 <= 7:
                fillers += kq_thunks(p + 2)
            if p >= 1:
                fillers += pv_thunks(p - 1)
            ptj = []
            nf = 0
            for g in range(8):
                h, gg = g // 4, g % 4
                ps2 = spp.tile([P, 1024], F32, tag="ps2", name="ps2")
                for half in range(2):
                    cj = 2 * gg + half
                    nc.tensor.matmul(
                        ps2[:, half * 512:(half + 1) * 512],
                        ktc[p][64 * h:64 * h + 64, cj * P:(cj + 1) * P],
                        qtc[p][64 * h:64 * h + 64, :],
                        start=True, stop=True, skip_group_check=True)
                pt2 = ptp.tile([P, 1024], BF16, tag="pt", name="pt2")
                nc.scalar.activation(pt2[:], ps2[:], AF.Exp)
                ptj.append(pt2)
                # interleave filler matmuls to keep the PE dense while the
                # scalar engine drains the exp queue
                take = (len(fillers) * (g + 1)) // 8 - nf
                for t in fillers[nf:nf + take]:
                    t()
                nf += take
            state[p] = ptj
            if p >= 1:
                pv_finish(p - 1)
        for t in pv_thunks(7):
            t()
        pv_finish(7)

    kv_cm.__exit__(None, None, None)
    wkq_cm.__exit__(None, None, None)
    hT_cm.__exit__(None, None, None)

    # ---- stage 4: flipped attn-out proj (token-major psum) + residual + LN2 ----
    x2s_cm = tc.tile_pool(name="x2s", bufs=1, side="right")
    x2s = x2s_cm.__enter__()
    x2 = [x2s.tile([P, E], F32, tag=f"x2_{i}", name=f"x2_{i}") for i in range(4)]
    h2T = [x2s.tile([P, TQ], BF16, tag=f"h2T{j}", name=f"h2T{j}") for j in range(8)]
    b2b = x2s.tile([P, E], F32, tag="b2b", name="b2b")
    nc.sync.dma_start(out=b2b[:], in_=d["b2b"][:, :])

    with tc.tile_pool(name="s4x", side="left", bufs=2) as xqp, \
         tc.tile_pool(name="s4s", side="left", bufs=6) as sp, \
         tc.tile_pool(name="ps_pj", bufs=3, space="PSUM") as ppp, \
         tc.tile_pool(name="ps_t4", bufs=4, space="PSUM") as tpp:
        for i in range(4):
            # xq = x(own) + bp_eff, prepared on gpsimd while PE runs proj
            xq = xqp.tile([P, E], F32, tag="xq")
            nc.sync.dma_start(out=xq[:], in_=d["x"][i * P:(i + 1) * P, :])
            nc.vector.scalar_tensor_tensor(xq[:], xq[:], 1.0, bpb[:],
                                           ALU.mult, ALU.add)
            for n in range(2):
                pj = ppp.tile([P, 512], F32, tag="pj", name="pj")
                for p in range(8):
                    nc.tensor.matmul(pj[:], otc[p][:, i * P:(i + 1) * P],
                                     wps[p][:, n * 512:(n + 1) * 512],
                                     start=(p == 0), stop=(p == 7))
                nc.vector.scalar_tensor_tensor(
                    x2[i][:, n * 512:(n + 1) * 512], pj[:], 1.0,
                    xq[:, n * 512:(n + 1) * 512], ALU.mult, ALU.add)
            stats = sp.tile([P, 2, 6], F32, tag="stats")
            nc.vector.bn_stats(stats[:, 0, :], x2[i][:, 0:512])
            nc.vector.bn_stats(stats[:, 1, :], x2[i][:, 512:1024])
            mv = sp.tile([P, 2], F32, tag="mv")
            nc.vector.bn_aggr(mv[:], stats[:])
            rsig = sp.tile([P, 1], F32, tag="rsig")
            nc.scalar.activation(rsig[:], mv[:, 1:2], AF.Sqrt, bias=epst[:])
            nc.vector.reciprocal(rsig[:], rsig[:])
            nmr = sp.tile([P, 1], F32, tag="nmr")
            nc.vector.scalar_tensor_tensor(nmr[:], mv[:, 0:1], -1.0, rsig[:],
                                           ALU.mult, ALU.mult)
            xn = sp.tile([P, E], BF16, tag="xn")
            nc.scalar.activation(xn[:], x2[i][:], AF.Identity,
                                 bias=nmr[:], scale=rsig[:])
            for j in range(8):
                pt = tpp.tile([P, P], BF16, tag="tp")
                nc.tensor.transpose(pt[:], xn[:, j * P:(j + 1) * P], eye[:])
                dst = h2T[j][:, i * P:(i + 1) * P]
                if j % 2 == 0:
                    nc.scalar.copy(dst, pt[:])
                else:
                    nc.vector.tensor_copy(dst, pt[:])

    ot_cm.__exit__(None, None, None)

    # ---- stage 5: FFN1 (feature-major rr) ----
    rr_cm = tc.tile_pool(name="relu", side="left", bufs=1)
    rrp = rr_cm.__enter__()
    rr = [rrp.tile([P, TQ], BF16, tag=f"r{k}", name=f"r{k}") for k in range(32)]
    # stage all of W2 during FFN1
    w2_cm = tc.tile_pool(name="w2s", side="left", bufs=1)
    w2p = w2_cm.__enter__()
    w2s = []
    for k in range(32):
        w2t = w2p.tile([P, E], BF16, tag=f"w2_{k}", name=f"w2_{k}")
        nc.sync.dma_start(out=w2t[:], in_=d["w2"][k * P:(k + 1) * P, :])
        w2s.append(w2t)

    with tc.tile_pool(name="ps_f1", bufs=4, space="PSUM") as fpp:
        for g in range(4):
            ws = w1g0 if g == 0 else load_w1_group(g)
            for m in range(8):
                ps = fpp.tile([P, TQ], F32, tag="ps")
                for k in range(8):
                    nc.tensor.matmul(ps[:], ws[k][:, m * P:(m + 1) * P],
                                     h2T[k][:], start=(k == 0), stop=(k == 7))
                col = g * 8 + m
                nc.scalar.activation(rr[col][:], ps[:], AF.Relu,
                                     bias=bias["b1"][:, col:col + 1])

    # ---- FFN2 flipped: token-major psum, fused residual + b2, streamed out ----
    with tc.tile_pool(name="outp", side="left", bufs=3) as outp, \
         tc.tile_pool(name="ps_f2", bufs=3, space="PSUM") as f2pp:
        for i in range(4):
            for n in range(2):
                ps = f2pp.tile([P, 512], F32, tag="pf", name="pf")
                for k in range(32):
                    nc.tensor.matmul(ps[:], rr[k][:, i * P:(i + 1) * P],
                                     w2s[k][:, n * 512:(n + 1) * 512],
                                     start=(k == 0), stop=(k == 31))
                ot = outp.tile([P, 512], F32, tag="ot", name="ot")
                nc.vector.scalar_tensor_tensor(
                    ot[:], ps[:], 1.0, x2[i][:, n * 512:(n + 1) * 512],
                    ALU.mult, ALU.add)
                nc.vector.scalar_tensor_tensor(
                    ot[:], ot[:], 1.0, b2b[:, n * 512:(n + 1) * 512],
                    ALU.mult, ALU.add)
                nc.sync.dma_start(
                    out=out_d[i * P:(i + 1) * P, n * 512:(n + 1) * 512],
                    in_=ot[:])

    w2_cm.__exit__(None, None, None)
    rr_cm.__exit__(None, None, None)
    w1_cm.__exit__(None, None, None)
    x2s_cm.__exit__(None, None, None)
    const_cm.__exit__(None, None, None)


def _build():
    nc = bacc.Bacc("TRN2", target_bir_lowering=False, debug=False,
                   num_devices=NCORES)
    d = {}

    def din(name, shape, dt=F32):
        d[name] = nc.dram_tensor(name, shape, dt, kind="ExternalInput").ap()

    din("x", [TK, E], F32)
    din("xbf", [TK, E], BF16)
    for n in ("wq", "wk", "wv", "wp"):
        din(n, [E, E], BF16)
    din("w1", [E, FF], BF16)
    din("w2", [FF, E], BF16)
    din("eye", [P, P], BF16)
    din("qb", [P, 8])
    din("kb", [P, 8])
    din("b1", [P, 32])
    din("bpb", [P, E])
    din("b2b", [P, E])
    out_d = nc.dram_tensor("out", [TQ, E], F32, kind="ExternalOutput").ap()
    with nc.allow_low_precision(reason="bf16 compute"):
        with tile.TileContext(nc) as tc:
            _emit(nc, tc, d, out_d)
    nc.compile()
    return nc


def _get_nc():
    if "nc" not in _CACHE:
        _CACHE["nc"] = _build()
    return _CACHE["nc"]


def _colmajor(v, width):
    return np.ascontiguousarray(np.asarray(v, np.float32).reshape(width, P).T)


def make_in_maps(x, ln1_g, ln1_b, Wq, Wk, Wv, Wp, bp, ln2_g, ln2_b,
                 W1, b1, W2, b2):
    x = np.asarray(x, dtype=np.float32)
    f32 = np.float32
    g1 = np.asarray(ln1_g, f32)
    b1n = np.asarray(ln1_b, f32)
    g2 = np.asarray(ln2_g, f32)
    b2n = np.asarray(ln2_b, f32)
    Wq_r = np.transpose(np.asarray(Wq, f32), (1, 0, 2)).reshape(E, E)
    Wk_r = np.transpose(np.asarray(Wk, f32), (1, 0, 2)).reshape(E, E)
    Wv_r = np.transpose(np.asarray(Wv, f32), (1, 0, 2)).reshape(E, E)
    Wp_ = np.asarray(Wp, f32)
    W1_ = np.asarray(W1, f32)
    qb = b1n @ Wq_r            # [E]
    kb = b1n @ Wk_r
    vb = b1n @ Wv_r
    bp_eff = np.asarray(bp, f32) + vb @ Wp_
    b1_eff = np.asarray(b1, f32) + b2n @ W1_
    shared = {
        "wq": np.ascontiguousarray(g1[:, None] * Wq_r).astype(ml_dtypes.bfloat16),
        "wk": np.ascontiguousarray(g1[:, None] * Wk_r).astype(ml_dtypes.bfloat16),
        "wv": np.ascontiguousarray(g1[:, None] * Wv_r).astype(ml_dtypes.bfloat16),
        "wp": Wp_.astype(ml_dtypes.bfloat16),
        "w1": np.ascontiguousarray(g2[:, None] * W1_).astype(ml_dtypes.bfloat16),
        "w2": np.asarray(W2, f32).astype(ml_dtypes.bfloat16),
        "eye": np.eye(P, dtype=ml_dtypes.bfloat16),
        "qb": _colmajor(qb, 8),
        "kb": _colmajor(kb, 8),
        "b1": _colmajor(b1_eff, 32),
        "bpb": np.ascontiguousarray(np.broadcast_to(bp_eff, (P, E))).astype(f32),
        "b2b": np.ascontiguousarray(
            np.broadcast_to(np.asarray(b2, f32), (P, E))).astype(f32),
    }
    in_maps = []
    for c in range(NCORES):
        b = c // 2
        q0 = TQ * (c % 2)
        xb = x[b]
        x_roll = np.ascontiguousarray(np.concatenate([xb[q0:], xb[:q0]], axis=0))
        in_maps.append({"x": x_roll,
                        "xbf": x_roll.astype(ml_dtypes.bfloat16), **shared})
    return in_maps


def assemble_out(results):
    out = np.empty((B, T, E), dtype=np.float32)
    for c in range(NCORES):
        b = c // 2
        q0 = TQ * (c % 2)
        out[b, q0:q0 + TQ] = results[c]["out"]
    return out


def kernel(x, ln1_g, ln1_b, Wq, Wk, Wv, Wp, bp, ln2_g, ln2_b, W1, b1, W2, b2,
           **_ignored):
    in_maps = make_in_maps(x, ln1_g, ln1_b, Wq, Wk, Wv, Wp, bp,
                           ln2_g, ln2_b, W1, b1, W2, b2)
    nc = _get_nc()
    res = run_bass_kernel_spmd(nc, in_maps, core_ids=list(range(NCORES)))
    return assemble_out(res.results)
